# revision 16
# baseline (speedup 1.0000x reference)
"""Trainium2 Bass kernel for causal GQA self-attention (fused QKV + RoPE).

Problem: B=2, T=2048, C=2048, H=16 q-heads, KV=4 kv-heads, HD=128.
Sharding: 8 cores = (batch b, kv-group k). Each core computes the 4 q-heads
of one kv group for one batch element; outputs are disjoint slices of y.

Per-core device kernel (bf16 matmuls, fp32 PSUM accumulation):
  1. QKV projection qkv^T = W_shard @ x^T, d-major layout [j, t].
     Host pre-transposes x and W into per-(quarter, t-block) contiguous
     layouts so every DMA moves 4-6KB contiguous per partition, and
     pre-permutes q/k head dims so RoPE becomes rotate-half. All bf16.
  2. RoPE on q/k via SBUF->SBUF DMA partition swap + DVE mul/add (bf16).
  3. Attention in S^T orientation: scores^T[s,t] = K^T.T @ Q^T. Score
     s-chunks are computed in PAIRS into 2-bank [128, 1024] PSUM tiles and
     exponentiated by a single 1024-wide ScalarE ACT (halves ACT overhead);
     causal diagonal masked by a triangular multiply on the valid slices
     (garbage columns of trimmed diagonal chunks are never read). Row sums:
     the 4 exp chunks of each group are combined on DVE, then one all-ones
     stationary matmul per group. PV accumulates y^T[d,t] in PSUM (fp32),
     written out as bf16.
  4. Projection tt=k is software-pipelined with attention t-block k-1.
     PSUM banks: proj accumulators 2 (j-blocks in 3 groups of 2, V-transpose
     shares the tag), attention y/sum 2, paired score tiles 2x2.
Output per core: unnormalized y^T [512, 2048] bf16 + row sums [16, 512] f32;
the host divides, transposes and concatenates.
"""

import math

import numpy as np
import ml_dtypes

import concourse.bass as bass
import concourse.mybir as mybir
import concourse.tile as tile
from concourse import bacc
from concourse.bass_utils import run_bass_kernel_spmd

B, T, C = 2, 2048, 2048
H, KV, HD = 16, 4, 128
NREP = H // KV  # q heads per core
P = 128
NCORES = 8
CC_CHUNKS = C // P  # 16 contraction chunks
TT = 4  # t-blocks of 512
TB = T // TT  # 512
NB = 6  # j-blocks per core: q0..q3, k, v
SCALE = 1.0 / math.sqrt(HD)

f32 = mybir.dt.float32
bf16 = mybir.dt.bfloat16

TRACE = False  # set True (with ntff shim installed) to get exec_time_ns

_cache = {}


def _build():
    if "nc" in _cache:
        return _cache["nc"]

    nc = bacc.Bacc("TRN2", target_bir_lowering=False, debug=False,
                   num_devices=NCORES)

    # x^T in per-(cq, tt) contiguous blocks: [P, cq, tt, ci, TB]
    xT_d = nc.dram_tensor("xT", [P, 4, TT, 4, TB], bf16,
                          kind="ExternalInput").ap()
    # W^T in per-quarter contiguous blocks: [P, wq, ci, NB*P]
    wT_d = nc.dram_tensor("wT", [P, 4, 4, NB * P], bf16,
                          kind="ExternalInput").ap()
    cc_d = nc.dram_tensor("CC", [P, T], bf16, kind="ExternalInput").ap()
    ss_d = nc.dram_tensor("SS2", [P, T], bf16, kind="ExternalInput").ap()
    tri_d = nc.dram_tensor("tri", [P, P], bf16, kind="ExternalInput").ap()
    ones_d = nc.dram_tensor("ones", [P, P], bf16, kind="ExternalInput").ap()
    ident_d = nc.dram_tensor("ident", [P, P], bf16, kind="ExternalInput").ap()
    yT_d = nc.dram_tensor("yT", [NREP * P, T], bf16, kind="ExternalOutput").ap()
    sums_d = nc.dram_tensor("sums", [NREP * TT, TB], f32, kind="ExternalOutput").ap()

    with tile.TileContext(nc) as tc:
        with (
            tc.tile_pool(name="wt", bufs=1) as wt_pool,
            tc.tile_pool(name="xt", bufs=5) as xt_pool,
            tc.tile_pool(name="qkvt", bufs=1) as qkv_pool,
            tc.tile_pool(name="freq", bufs=1) as freq_pool,
            tc.tile_pool(name="small", bufs=1) as small_pool,
            tc.tile_pool(name="vsb", bufs=1) as v_pool,
            tc.tile_pool(name="swp", bufs=2) as swp_pool,
            tc.tile_pool(name="ropetmp", bufs=2) as rt_pool,
            tc.tile_pool(name="exp2", bufs=4) as exp2_pool,
            tc.tile_pool(name="ecum", bufs=2) as ec_pool,
            tc.tile_pool(name="yout", bufs=2) as y_pool,
            tc.tile_pool(name="psproj", bufs=2, space="PSUM") as psproj_pool,
            tc.tile_pool(name="psacc", bufs=2, space="PSUM") as psacc_pool,
            tc.tile_pool(name="pss2", bufs=2, space="PSUM") as pss2_pool,
        ):
            # ---- resident tensors ----
            wt_q = [
                wt_pool.tile([P, 4, NB * P], bf16, tag=f"wt{wq}", name=f"wt{wq}")
                for wq in range(4)
            ]
            wt_loaded = [False] * 4

            # qkv^T blocks [128 d, 2048 t]: jb 0..3 = q heads (rope-permuted),
            # 4 = k (rope-permuted), 5 = v
            qkvT = [
                qkv_pool.tile([P, T], bf16, tag=f"qkv{jb}", name=f"qkv{jb}")
                for jb in range(NB)
            ]
            # V in s-major: [128 s, 16 s-chunk, 128 d]
            v_sb = v_pool.tile([P, CC_CHUNKS, P], bf16, tag="vsb")

            # late-loaded constants (needed only after proj tt0 matmuls)
            ccs = freq_pool.tile([P, T], bf16, tag="cc")
            ss2 = freq_pool.tile([P, T], bf16, tag="ss")
            tri = small_pool.tile([P, P], bf16, tag="tri")
            ones = small_pool.tile([P, P], bf16, tag="ones")
            ident = small_pool.tile([P, P], bf16, tag="ident")
            _late = [(ccs, cc_d), (ss2, ss_d), (tri, tri_d), (ones, ones_d),
                     (ident, ident_d)]

            # xt tiles for a tt block (loaded in the first group, reused)
            xt_tiles = {}

            def proj_mms(tt, jbs, psums):
                """Projection matmuls for j-blocks `jbs` of t-block tt."""
                for cq in range(4):
                    finegrain = tt == 0 and cq == 0
                    key = (tt, cq)
                    if finegrain:
                        # critical path to the first matmul: alternate w/x
                        # per c-chunk on the two HWDGE queues (sync + scalar)
                        xt = xt_pool.tile([P, 4, TB], bf16, tag="xt", name="xt")
                        for ci in range(4):
                            nc.sync.dma_start(
                                wt_q[cq][:, ci:ci + 1, :],
                                wT_d[:, 0, ci:ci + 1, :],
                            )
                            nc.scalar.dma_start(
                                xt[:, ci:ci + 1, :],
                                xT_d[:, 0, 0, ci:ci + 1, :],
                            )
                        wt_loaded[cq] = True
                        xt_tiles[key] = xt
                    else:
                        if not wt_loaded[cq]:
                            nc.sync.dma_start(wt_q[cq][:], wT_d[:, cq])
                            wt_loaded[cq] = True
                        if key not in xt_tiles:
                            xt = xt_pool.tile([P, 4, TB], bf16, tag="xt",
                                              name="xt")
                            nc.scalar.dma_start(xt[:], xT_d[:, cq, tt])
                            xt_tiles[key] = xt
                    xt = xt_tiles[key]
                    for ci in range(4):
                        cc = cq * 4 + ci
                        for j, jb in enumerate(jbs):
                            nc.tensor.matmul(
                                psums[j][:],
                                wt_q[cq][:, ci, jb * P:(jb + 1) * P],
                                xt[:, ci, :],
                                start=(cc == 0),
                                stop=(cc == CC_CHUNKS - 1),
                            )

            def proj_post(tt, jbs, psums):
                """PSUM->SBUF copies, RoPE (q/k) and V transpose for `jbs`."""
                tsl = slice(tt * TB, (tt + 1) * TB)
                for j, jb in enumerate(jbs):
                    if jb % 2 == 0:
                        nc.vector.tensor_copy(qkvT[jb][:, tsl], psums[j][:])
                    else:
                        nc.scalar.copy(qkvT[jb][:, tsl], psums[j][:])
                for jb in jbs:
                    if jb == 5:
                        # V transpose for this chunk: v^T [d, s] -> v_sb [s, d]
                        for i in range(4):
                            sc = 4 * tt + i
                            trp = psproj_pool.tile([P, TB], bf16, tag="proj",
                                                   name="trp")
                            nc.tensor.transpose(
                                trp[:, :P], qkvT[5][:, sc * P:(sc + 1) * P],
                                ident[:]
                            )
                            nc.vector.tensor_copy(v_sb[:, sc, :], trp[:, :P])
                    else:
                        # RoPE on this t-chunk
                        swp = swp_pool.tile([P, TB], bf16, tag="swp", name="swp")
                        nc.sync.dma_start(swp[0:64, :], qkvT[jb][64:128, tsl])
                        nc.sync.dma_start(swp[64:128, :], qkvT[jb][0:64, tsl])
                        ta = rt_pool.tile([P, TB], bf16, tag="ta", name="ta")
                        tb_ = rt_pool.tile([P, TB], bf16, tag="tb", name="tb")
                        nc.vector.tensor_tensor(
                            ta[:], qkvT[jb][:, tsl], ccs[:, tsl],
                            mybir.AluOpType.mult
                        )
                        nc.vector.tensor_tensor(
                            tb_[:], swp[:], ss2[:, tsl], mybir.AluOpType.mult
                        )
                        nc.vector.tensor_tensor(
                            qkvT[jb][:, tsl], ta[:], tb_[:], mybir.AluOpType.add
                        )

            def attn_head(tb, h):
                """Attention for (t-block tb, head h), S^T orientation.

                Score chunks are computed in pairs into 2-bank PSUM tiles and
                exp'd with one 1024-wide ACT. For trimmed diagonal chunks the
                columns below the causal block are garbage but never read."""
                psum_y = psacc_pool.tile([P, TB], f32, tag="acc", name="psum_y")
                psum_sum = psacc_pool.tile([P, TB], f32, tag="acc",
                                           name="psum_sum")
                nsc = 4 * (tb + 1)
                ngroups = tb + 1
                for g in range(ngroups):
                    diag = g == tb
                    eslices = []
                    for half in range(2):
                        ps2 = pss2_pool.tile([P, 2 * TB], f32, tag="s2",
                                             name="ps2")
                        e2 = exp2_pool.tile([P, 2 * TB], bf16, tag="e2",
                                            name="e2")
                        for i2 in range(2):
                            sc = 4 * g + 2 * half + i2
                            r = sc - 4 * tb  # >=0: diagonal-crossing chunk
                            col0 = r * P if r >= 0 else 0
                            nc.tensor.matmul(
                                ps2[:, i2 * TB + col0:(i2 + 1) * TB],
                                qkvT[4][:, sc * P:(sc + 1) * P],
                                qkvT[h][:, tb * TB + col0:(tb + 1) * TB],
                                start=True,
                                stop=True,
                            )
                        nc.scalar.activation(
                            e2[:],
                            ps2[:],
                            mybir.ActivationFunctionType.Exp,
                            scale=SCALE,
                        )
                        for i2 in range(2):
                            sc = 4 * g + 2 * half + i2
                            r = sc - 4 * tb
                            col0 = r * P if r >= 0 else 0
                            esl = e2[:, i2 * TB + col0:(i2 + 1) * TB]
                            if r >= 0:
                                nc.vector.tensor_tensor(
                                    e2[:, i2 * TB + col0:i2 * TB + col0 + P],
                                    e2[:, i2 * TB + col0:i2 * TB + col0 + P],
                                    tri[:],
                                    mybir.AluOpType.mult,
                                )
                            nc.tensor.matmul(
                                psum_y[:, col0:],
                                v_sb[:, sc, :],
                                esl,
                                start=(sc == 0),
                                stop=(sc == nsc - 1),
                            )
                            eslices.append((e2, i2 * TB, col0))
                    # combine the group's 4 exp chunks on DVE, 1 sum matmul
                    ec = ec_pool.tile([P, TB], bf16, tag="ec", name="ec")
                    (ta_, oa, _), (tb2, ob, _), (tc_, oc, _), (td, od, _) = \
                        eslices
                    if not diag:
                        ec2 = ec_pool.tile([P, TB], bf16, tag="ec2", name="ec2")
                        nc.vector.tensor_tensor(
                            ec[:], ta_[:, oa:oa + TB], tb2[:, ob:ob + TB],
                            mybir.AluOpType.add
                        )
                        nc.vector.tensor_tensor(
                            ec2[:], tc_[:, oc:oc + TB], td[:, od:od + TB],
                            mybir.AluOpType.add
                        )
                        nc.vector.tensor_tensor(
                            ec[:], ec[:], ec2[:], mybir.AluOpType.add
                        )
                    else:
                        # chunk k valid from col k*P
                        nc.vector.tensor_copy(
                            ec[:, 0:P], ta_[:, oa:oa + P]
                        )
                        nc.vector.tensor_tensor(
                            ec[:, P:], ta_[:, oa + P:oa + TB],
                            tb2[:, ob + P:ob + TB], mybir.AluOpType.add
                        )
                        nc.vector.tensor_tensor(
                            ec[:, 2 * P:], ec[:, 2 * P:],
                            tc_[:, oc + 2 * P:oc + TB], mybir.AluOpType.add
                        )
                        nc.vector.tensor_tensor(
                            ec[:, 3 * P:], ec[:, 3 * P:],
                            td[:, od + 3 * P:od + TB], mybir.AluOpType.add
                        )
                    nc.tensor.matmul(
                        psum_sum[:],
                        ones[:],
                        ec[:],
                        start=(g == 0),
                        stop=(g == ngroups - 1),
                    )
                y_sb = y_pool.tile([P, TB], bf16, tag="ysb", name="ysb")
                nc.vector.tensor_copy(y_sb[:], psum_y[:])
                nc.sync.dma_start(
                    yT_d[h * P:(h + 1) * P, tb * TB:(tb + 1) * TB], y_sb[:]
                )
                sums_sb = y_pool.tile([1, TB], f32, tag="sums", name="sums_sb")
                nc.vector.tensor_copy(sums_sb[:], psum_sum[0:1, :])
                nc.sync.dma_start(
                    sums_d[h * TT + tb:h * TT + tb + 1, :], sums_sb[0:1, :]
                )

            # ---- software pipeline: proj(tt) overlapped with attn(tb=tt-1) ----
            JGROUPS = [[0, 1], [2, 3], [4, 5]]
            for step in range(TT + 1):
                tt = step if step < TT else None
                tb = step - 1 if step >= 1 else None
                if tb is not None:
                    attn_head(tb, 0)
                    attn_head(tb, 1)
                for gi, jbs in enumerate(JGROUPS if tt is not None else []):
                    ps = [psproj_pool.tile([P, TB], f32, tag="proj",
                                           name="proj_ps") for _ in jbs]
                    proj_mms(tt, jbs, ps)
                    if tt == 0 and gi == 0:
                        for _tile, _src in _late:
                            nc.sync.dma_start(_tile[:], _src[:])
                        _late = []
                    proj_post(tt, jbs, ps)
                    if tb is not None and gi < 2:
                        attn_head(tb, 2 + gi)
                if tt is None and tb is not None:
                    attn_head(tb, 2)
                    attn_head(tb, 3)
                if tt is not None:
                    for cq in range(4):
                        xt_tiles.pop((tt, cq), None)

    nc.compile()
    _cache["nc"] = nc
    return nc


def _host_prep(x, w_qkv, freqs_cos, freqs_sin):
    """Build per-core input maps (numpy, cheap)."""
    x = np.asarray(x, dtype=np.float32)
    w_qkv = np.asarray(w_qkv, dtype=np.float32)
    freqs_cos = np.asarray(freqs_cos, dtype=np.float32)
    freqs_sin = np.asarray(freqs_sin, dtype=np.float32)
    bf = ml_dtypes.bfloat16

    perm = np.concatenate([np.arange(0, HD, 2), np.arange(1, HD, 2)])

    xTs = []
    for b in range(B):
        # [C, T] -> [P, cq, tt, ci, TB]
        xt = x[b].T.reshape(4, 4, P, TT, TB).transpose(2, 0, 3, 1, 4)
        xTs.append(np.ascontiguousarray(xt).astype(bf))

    cosT = freqs_cos.T  # [64, T]
    sinT = freqs_sin.T
    CCh = np.concatenate([cosT, cosT], axis=0).astype(bf)
    SS2 = np.concatenate([-sinT, sinT], axis=0).astype(bf)
    tri = np.triu(np.ones((P, P), dtype=np.float32)).astype(bf)
    ones = np.ones((P, P), dtype=bf)
    ident = np.eye(P, dtype=np.float32).astype(bf)

    in_maps = []
    for core in range(NCORES):
        b, kv = divmod(core, KV)
        blocks = []
        for r in range(NREP):
            hrow = (kv * NREP + r) * HD
            blocks.append(w_qkv[hrow:hrow + HD][perm])
        blocks.append(w_qkv[H * HD + kv * HD:H * HD + (kv + 1) * HD][perm])
        blocks.append(
            w_qkv[(H + KV) * HD + kv * HD:(H + KV) * HD + (kv + 1) * HD]
        )
        w_shard = np.concatenate(blocks, axis=0)  # [768, C]
        # [C, 768] -> [P, wq, ci, 768]
        wT = w_shard.T.reshape(4, 4, P, NB * P).transpose(2, 0, 1, 3)
        wT = np.ascontiguousarray(wT).astype(bf)
        in_maps.append({
            "xT": xTs[b],
            "wT": wT,
            "CC": CCh,
            "SS2": SS2,
            "tri": tri,
            "ones": ones,
            "ident": ident,
        })
    return in_maps


def kernel(x, w_qkv, freqs_cos, freqs_sin):
    nc = _build()
    in_maps = _host_prep(x, w_qkv, freqs_cos, freqs_sin)
    res = run_bass_kernel_spmd(nc, in_maps, list(range(NCORES)), trace=TRACE)
    _cache["last_res"] = res

    y = np.empty((B, T, C), dtype=np.float32)
    for core in range(NCORES):
        b, kv = divmod(core, KV)
        yT = res.results[core]["yT"].astype(np.float32)  # [NREP*P, T] unnorm.
        sums = res.results[core]["sums"].reshape(NREP, T)  # per (h, t)
        yT = yT.reshape(NREP, P, T) / sums[:, None, :]
        y[b, :, kv * NREP * HD:(kv + 1) * NREP * HD] = (
            yT.reshape(NREP * P, T).T
        )
    return y
